# revision 67
# baseline (speedup 1.0000x reference)
"""Single-head attention (B=8, S=2048, H=768, D=64) on 8 TRN2 NeuronCores.

Data-parallel over batch: core b computes batch element b end to end; no
collectives. Host prepacks Q/K/V into SBUF-native [128, ...] bf16 layouts
(linear DMA: 128 big descriptors per transfer instead of 768 x 2KB lines)
and does the final softmax divide + transpose; the device ships
unnormalized O^T (bf16) plus denominators.

Design rules (from perfetto/NTFF iteration; see git history of the
optimization session for the evidence):
  - The ACT exp stream (32 x [128,1024] @ ~1.11us each, ~35.6us busy) is
    the pacing resource. Makespan ~= first-exp time + ACT busy + tail.
  - ACT is saturated after the first exp, so any PE insertion larger
    than the ~600ns per-period slack delays every later exp permanently:
    all projections are emitted as 2-3 h-granule slices, one per period.
  - The HAM clock gate closes after ~1us of PE idleness and then runs
    the WHOLE core (ACT included) ~20% slower for tens of us: 64 warmup
    matmuls bridge the DMA lead-in, and every phase keeps the PE dense.
    Never leave a >1us PE hole mid-kernel.
  - Input DMA rides the sync queue in exact consumption order
    k0 q0ab q1ab k1 k2 k3 q2 wv q3 v0 v1 (q chunks h-split so their
    projections start early); weights (wqk) ride the scalar
    queue first. DMA completion sems cost ~0.9us (SEM_PROP) on top of
    arrival, and per-transfer issue costs ~0.7us on the issuing engine.
  - Tiles 0,1 run half-width chunk-0 scores+exp so the exp stream starts
    before q chunk 1 is even projected.
  - Softmax denominators: GPSIMD (idle otherwise) sums pth tile pairs
    (~2.1us each, SBUF-only), PE runs 4x-col-packed quads on the pair
    sums two+ periods later; tiles 14,15 use direct quads in the tail.
    DVE must NOT touch the pth pool while ACT writes it (SBUF port
    contention adds ~220ns to every exp) - hence gpsimd.
  - O^T accumulates as av4: both halves of a tile back to back sharing
    one stationary load of the [sk,d] V tile; av half-0 of tiles 14,15
    is injected early so pav[0] stops at av4(13) and o1 ships before the
    last exp. PE-transposes (not DMA_TRANSPOSE: 1.2us issue each!) build
    the [sk,d] V tiles.
"""

import os
from contextlib import ExitStack

import numpy as np
import ml_dtypes

import concourse.bass as bass
import concourse.mybir as mybir
import concourse.tile as tile
from concourse import bacc
from concourse.bass_utils import run_bass_kernel_spmd

S, H, D = 2048, 768, 64
P = 128
NT = S // P      # 16 sk tiles
HT = H // P      # 6 h tiles
CH = 512         # sq chunk = matmul free dim = PSUM bank
NCH = S // CH    # 4
BF = mybir.dt.bfloat16
F32 = mybir.dt.float32
AF = mybir.ActivationFunctionType

LAST_RESULT = None  # BassKernelResults of the most recent run (for test.py)


def _build(debug=False):
    nc = bacc.Bacc()
    qpk_d = nc.declare_dram_parameter("qpk", [P, NCH * HT * CH], BF, isOutput=False)
    kpk_d = nc.declare_dram_parameter("kpk", [P, NCH * HT * CH], BF, isOutput=False)
    # v prepack: {wv | v half0 | v half1}
    VW = HT * 2 * CH  # 6144 elements per v half
    vpk_d = nc.declare_dram_parameter("vpk", [P, HT * D + 2 * VW], BF, isOutput=False)
    wqk_d = nc.declare_dram_parameter("wqk", [P, HT * P], BF, isOutput=False)
    ict_d = nc.declare_dram_parameter("ict", [P, P], BF, isOutput=False)
    cst_d = nc.declare_dram_parameter("cst", [P, 4 + NT], F32, isOutput=False)
    o1_d = nc.declare_dram_parameter("o1", [P, CH], BF, isOutput=True)
    o2_d = nc.declare_dram_parameter("o2", [P, CH], BF, isOutput=True)
    o3_d = nc.declare_dram_parameter("o3", [P, CH], BF, isOutput=True)

    with ExitStack() as ctx:
        tc = ctx.enter_context(tile.TileContext(nc))
        consts = ctx.enter_context(tc.tile_pool(name="consts", bufs=1))
        stages = ctx.enter_context(tc.tile_pool(name="stages", bufs=1))
        persist = ctx.enter_context(tc.tile_pool(name="persist", bufs=1))
        ppool = ctx.enter_context(tc.tile_pool(name="ppool", bufs=2 * NT))
        p2pool = ctx.enter_context(tc.tile_pool(name="p2pool", bufs=14))
        psc = ctx.enter_context(tc.tile_pool(name="psc", bufs=1, space="PSUM"))
        psw = ctx.enter_context(tc.tile_pool(name="psw", bufs=1, space="PSUM"))
        psav = ctx.enter_context(tc.tile_pool(name="psav", bufs=1, space="PSUM"))

        # ---- constants on the scalar queue: wqk first (critical path) ----
        wqk_sb = consts.tile([P, HT * P], BF, tag="wqk")
        nc.scalar.dma_start(out=wqk_sb, in_=wqk_d[:, :])
        cst_sb = consts.tile([P, 4 + NT], F32, tag="cst")
        nc.scalar.dma_start(out=cst_sb, in_=cst_d[:, :])
        ict_sb = consts.tile([P, P], BF, tag="ict")
        nc.scalar.dma_start(out=ict_sb, in_=ict_d[:, :])
        ident_bf = ict_sb
        bvv_sb = cst_sb[:, 1:2]
        bkk_sb = cst_sb[:, 2:3]
        bqq_sb = cst_sb[:, 3:4]
        mb_sb = cst_sb[:, 4 : 4 + NT]
        warmT = consts.tile([P, P], BF, tag="warmT")
        nc.vector.memset(warmT, 0.0)
        ones32 = consts.tile([P, 32], BF, tag="ones32")
        nc.vector.memset(ones32, 1.0)

        def w_qk(h, lo, hi):  # packed q|k weight slice [128, hi-lo]
            return wqk_sb[:, h * P + lo : h * P + hi]

        # ---- input staging: linear transfers on the sync queue in exact
        # consumption order k0,q0,q1,k1,v0(+wvv),k2,k3,v1,q2,q3 ----
        stq = stages.tile([P, NCH, HT * CH], BF, tag="stq")
        stk = stages.tile([P, NCH, HT * CH], BF, tag="stk")
        stv = stages.tile([P, 2, VW], BF, tag="stv")
        wvv_sb = stages.tile([P, HT * D], BF, tag="wvv")
        QW = HT * CH  # 3072 elements per k/q quarter

        def dma_kq(dst, src, c):
            nc.sync.dma_start(out=dst[:, c, :], in_=src[:, c * QW : (c + 1) * QW])

        dma_kq(stk, kpk_d, 0)
        # q chunks 0,1 split in h-halves so projections start ~1us earlier
        HQW = QW // 2
        nc.sync.dma_start(out=stq[:, 0, 0:HQW], in_=qpk_d[:, 0:HQW])
        nc.sync.dma_start(out=stq[:, 0, HQW:QW], in_=qpk_d[:, HQW:QW])
        nc.sync.dma_start(
            out=stq[:, 1, 0:HQW], in_=qpk_d[:, QW : QW + HQW]
        )
        nc.sync.dma_start(
            out=stq[:, 1, HQW:QW], in_=qpk_d[:, QW + HQW : 2 * QW]
        )
        dma_kq(stk, kpk_d, 1)
        dma_kq(stk, kpk_d, 2)
        dma_kq(stk, kpk_d, 3)
        dma_kq(stq, qpk_d, 2)
        WV0 = HT * D
        nc.sync.dma_start(out=wvv_sb, in_=vpk_d[:, 0:WV0])
        dma_kq(stq, qpk_d, 3)
        nc.sync.dma_start(out=stv[:, 0, :], in_=vpk_d[:, WV0 : WV0 + VW])
        nc.sync.dma_start(out=stv[:, 1, :], in_=vpk_d[:, WV0 + VW : WV0 + 2 * VW])

        def w_vv(h):
            return wvv_sb[:, h * D : (h + 1) * D]

        def st_kq(st, h, c):  # [128, 512] h-tile slice of chunk c
            return st[:, c, h * CH : (h + 1) * CH]

        def st_v(h, half, i):  # [128, 512] h-tile slice of chunk 2*half+i
            return stv[:, half, h * 2 * CH + i * CH : h * 2 * CH + (i + 1) * CH]

        # ---- persistent SBUF tensors ----
        qqT_sb = persist.tile([P, S], BF, tag="qqT")
        kkT_sb = persist.tile([P, S], BF, tag="kkT")
        vT2_sb = persist.tile([P, S // 2], BF, tag="vT2")
        vE_sb = persist.tile([P, NT * D], BF, tag="vE")
        osb = persist.tile([P, 3 * CH], BF, tag="osb")  # output staging

        # ---- PE warmup: bridge the whole DMA lead-in ----
        warm = psw.tile([P, CH], F32, tag="aux", name="warm")
        for i in range(64):
            nc.tensor.matmul(
                warm[:, :P],
                lhsT=warmT,
                rhs=warmT,
                start=True,
                stop=True,
                skip_group_check=True,
            )

        # ---- helper blocks ----
        pp_t = {}

        def kq1proj(kind, c, ptag, hs=(0, HT), pool=None):
            """single-chunk projection (kind 0 = k -> kkT, 1 = q -> qqT):
            col-packed concurrent pairs produce native+duplicated partition
            halves of chunk c (h range hs), then one [128,512] bias drain.
            Split across two calls to spread PE load across exp periods."""
            dst = kkT_sb if kind == 0 else qqT_sb
            bias = bkk_sb if kind == 0 else bqq_sb
            wlo = D if kind == 0 else 0
            stx = stk if kind == 0 else stq
            if (kind, c) not in pp_t:
                pp_t[kind, c] = (pool or psav).tile([P, CH], F32, tag=ptag,
                                                    name=f"pp{kind}_{c}")
            pp = pp_t[kind, c]
            for h in range(*hs):
                nc.tensor.matmul(
                    pp[:D, :],
                    lhsT=w_qk(h, wlo, wlo + D),
                    rhs=st_kq(stx, h, c),
                    start=(h == 0),
                    stop=(h == HT - 1),
                    tile_position=(0, 0),
                    skip_group_check=True,
                )
                nc.tensor.matmul(
                    pp[D:, :],
                    lhsT=w_qk(h, wlo, wlo + D),
                    rhs=st_kq(stx, h, c),
                    start=(h == 0),
                    stop=(h == HT - 1),
                    tile_position=(0, D),
                    skip_group_check=True,
                )
            if hs[1] == HT:
                nc.vector.tensor_scalar_add(
                    out=dst[:, c * CH : (c + 1) * CH], in0=pp, scalar1=bias
                )

        pth = {}

        def scores_exp(t, half):
            """scores for sk-tile t over sq half (row group alternates with
            t to hide LDWEIGHTS), exp straight into a bf16 tile."""
            lo, hi = (0, D) if t % 2 == 0 else (D, P)
            ps = psc.tile([P, 2 * CH], F32, tag=f"sc{t % 2}",
                          name=f"ps{t}_{half}")
            for sub in range(2):
                c = 2 * half + sub
                nc.tensor.matmul(
                    ps[:, sub * CH : (sub + 1) * CH],
                    lhsT=kkT_sb[lo:hi, t * P : (t + 1) * P],
                    rhs=qqT_sb[lo:hi, c * CH : (c + 1) * CH],
                    start=True,
                    stop=True,
                    tile_position=(lo, 0),
                    skip_group_check=True,
                )
            pt = ppool.tile([P, 2 * CH], BF, tag="pT", name=f"pt{t}_{half}")
            nc.scalar.activation(
                out=pt,
                in_=ps,
                func=AF.Exp,
                bias=mb_sb[:, t : t + 1],
                scale=0.125,
            )
            pth[t, half] = pt

        pav = {}

        def vE_slice(t):
            b = (t // 8) * 4 + (t % 4)
            half = (t % 8) // 4
            return vE_sb[:, b * P + half * D : b * P + half * D + D]

        def av_h(t, half, start, stop):
            """col-packed concurrent O^T accumulation pair for sk-tile t,
            one half: chunk 2h -> partitions 0:64, chunk 2h+1 -> 64:128."""
            key = "av23" if half == 0 else "av01"
            if half not in pav:
                pav[half] = psav.tile([P, CH], F32, tag=key, name=f"pav{half}")
            vt = vE_slice(t)
            nc.tensor.matmul(
                pav[half][:D, :],
                lhsT=vt,
                rhs=pth[t, half][:, :CH],
                start=start,
                stop=stop,
                tile_position=(0, 0),
                skip_group_check=True,
            )
            nc.tensor.matmul(
                pav[half][D:, :],
                lhsT=vt,
                rhs=pth[t, half][:, CH:],
                start=start,
                stop=stop,
                tile_position=(0, D),
                skip_group_check=True,
            )

        def av4(t):
            """both halves of tile t back to back (shared stationary vt);
            pav0 accumulation order is 0..13 then 14,15 early-injected, so
            start/stop flags are passed per call site."""
            av_h(t, 0, start=(t == 0), stop=(t == 13))
            av_h(t, 1, start=(t == 0), stop=(t == NT - 1))

        pden = [None]

        def _den_quad(rhs_of_ci, start, stop):
            if pden[0] is None:
                pden[0] = psw.tile([P, CH], F32, tag="den", name="pden")
            for ci in range(NCH):
                nc.tensor.matmul(
                    pden[0][32 * ci : 32 * (ci + 1), :],
                    lhsT=ones32[:, :],
                    rhs=rhs_of_ci(ci),
                    start=start,
                    stop=stop,
                    tile_position=(0, 32 * ci),
                    skip_group_check=True,
                )

        p2 = {}

        def pair_sum(k, half):
            """GPSIMD: p2[k,half] = pth[2k,half] + pth[2k+1,half]
            (SBUF only; keeps the denominator adds off PE and DVE)."""
            t2 = p2pool.tile([P, 2 * CH], BF, tag="p2", name=f"p2_{k}_{half}")
            nc.gpsimd.tensor_add(
                out=t2, in0=pth[2 * k, half], in1=pth[2 * k + 1, half]
            )
            p2[k, half] = t2

        def den4q(k):
            """denominator quad on pair-sum tiles (covers tiles 2k, 2k+1)."""
            _den_quad(
                lambda ci: p2[k, ci // 2][:, (ci % 2) * CH : (ci % 2 + 1) * CH],
                start=(k == 0),
                stop=False,
            )

        def den4(t, stop=False):
            """denominator quad on raw pth tiles (used for tiles 12-15)."""
            _den_quad(
                lambda ci: pth[t, ci // 2][:, (ci % 2) * CH : (ci % 2 + 1) * CH],
                start=False,
                stop=stop,
            )

        pv_t = {}

        def v_proj_mm(u, hs=(0, HT)):
            """v projection matmuls (h range hs) + bias for chunk-pair u:
            chunk 2u -> partitions 0:64, chunk 2u+1 -> 64:128 (stacked)."""
            if u not in pv_t:
                pv_t[u] = psw.tile([P, CH], F32,
                                   tag="den" if u == 0 else "aux",
                                   name=f"pv{u}")
            pv = pv_t[u]
            for h in range(*hs):
                nc.tensor.matmul(
                    pv[:D, :],
                    lhsT=w_vv(h),
                    rhs=st_v(h, u, 0),
                    start=(h == 0),
                    stop=(h == HT - 1),
                    tile_position=(0, 0),
                    skip_group_check=True,
                )
                nc.tensor.matmul(
                    pv[D:, :],
                    lhsT=w_vv(h),
                    rhs=st_v(h, u, 1),
                    start=(h == 0),
                    stop=(h == HT - 1),
                    tile_position=(0, D),
                    skip_group_check=True,
                )
            if hs[1] == HT:
                nc.vector.tensor_scalar_add(
                    out=vT2_sb[:, u * CH : (u + 1) * CH], in0=pv,
                    scalar1=bvv_sb,
                )

        def v_transpose(u, j):
            """one [128,128] PE-transpose block of vT2 into vE
            (block b = 4u+j holds sk tiles 8u+j and 8u+j+4)."""
            pt = psw.tile([P, P], BF, tag="den" if u == 0 else "aux",
                          name=f"ptv{u}_{j}")
            nc.tensor.transpose(
                pt,
                in_=vT2_sb[:, u * CH + j * P : u * CH + (j + 1) * P],
                identity=ident_bf,
            )
            b = 4 * u + j
            nc.vector.tensor_copy(out=vE_sb[:, b * P : (b + 1) * P], in_=pt)

        # ---- phase A schedule: every PE insertion is split to fit
        # per-exp-period slack (the ACT stream is saturated; any larger
        # insertion delays every later exp permanently) ----
        kq1proj(0, 0, "av01")            # k chunk 0 (arrives first)
        kq1proj(1, 0, "av23", (0, 3))    # q chunk 0 (h-split transfer)
        kq1proj(1, 0, "av23", (3, HT))
        # tiles 0,1: half-width scores+exp on chunk 0 only, so the exp
        # stream starts ~2us before q chunk 1 is even projected
        ps01 = {}
        for t in (0, 1):
            lo, hi = (0, D) if t % 2 == 0 else (D, P)
            ps01[t] = psc.tile([P, 2 * CH], F32, tag=f"sc{t % 2}",
                               name=f"ps{t}_0")
            pth[t, 0] = ppool.tile([P, 2 * CH], BF, tag="pT", name=f"pt{t}_0")
            nc.tensor.matmul(
                ps01[t][:, 0:CH],
                lhsT=kkT_sb[lo:hi, t * P : (t + 1) * P],
                rhs=qqT_sb[lo:hi, 0:CH],
                start=True, stop=True,
                tile_position=(lo, 0),
                skip_group_check=True,
            )
        # tile 2 chunk-0 rides the otherwise-free "den" PSUM bank so a
        # third early exp fills part of the q-chunk-1 wait
        ps2a = psw.tile([P, CH], F32, tag="den", name="ps2a")
        pth[2, 0] = ppool.tile([P, 2 * CH], BF, tag="pT", name="pt2_0")
        nc.tensor.matmul(
            ps2a,
            lhsT=kkT_sb[0:D, 2 * P : 3 * P],
            rhs=qqT_sb[0:D, 0:CH],
            start=True, stop=True,
            tile_position=(0, 0),
            skip_group_check=True,
        )
        for t in (0, 1):
            nc.scalar.activation(
                out=pth[t, 0][:, 0:CH], in_=ps01[t][:, 0:CH],
                func=AF.Exp, bias=mb_sb[:, t : t + 1], scale=0.125,
            )
        nc.scalar.activation(
            out=pth[2, 0][:, 0:CH], in_=ps2a,
            func=AF.Exp, bias=mb_sb[:, 2:3], scale=0.125,
        )
        kq1proj(1, 1, "aux", (0, 3), pool=psw)   # q chunk 1 (h-split)
        kq1proj(1, 1, "aux", (3, HT), pool=psw)
        ps2b = psc.tile([P, 2 * CH], F32, tag="sc0", name="ps2_0")
        for t in (0, 1):
            lo, hi = (0, D) if t % 2 == 0 else (D, P)
            nc.tensor.matmul(
                ps01[t][:, CH : 2 * CH],
                lhsT=kkT_sb[lo:hi, t * P : (t + 1) * P],
                rhs=qqT_sb[lo:hi, CH : 2 * CH],
                start=True, stop=True,
                tile_position=(lo, 0),
                skip_group_check=True,
            )
        nc.tensor.matmul(
            ps2b[:, CH : 2 * CH],
            lhsT=kkT_sb[0:D, 2 * P : 3 * P],
            rhs=qqT_sb[0:D, CH : 2 * CH],
            start=True, stop=True,
            tile_position=(0, 0),
            skip_group_check=True,
        )
        for t in (0, 1):
            nc.scalar.activation(
                out=pth[t, 0][:, CH : 2 * CH], in_=ps01[t][:, CH : 2 * CH],
                func=AF.Exp, bias=mb_sb[:, t : t + 1], scale=0.125,
            )
        nc.scalar.activation(
            out=pth[2, 0][:, CH : 2 * CH], in_=ps2b[:, CH : 2 * CH],
            func=AF.Exp, bias=mb_sb[:, 2:3], scale=0.125,
        )
        kq1proj(0, 1, "av23", (0, 3))    # k chunk 1
        pair_sum(0, 0)
        kq1proj(0, 1, "av23", (3, HT))
        scores_exp(3, 0)
        kq1proj(0, 2, "av01", (0, 3))    # k chunk 2
        scores_exp(4, 0)
        pair_sum(1, 0)
        kq1proj(0, 2, "av01", (3, HT))
        scores_exp(5, 0)
        kq1proj(0, 3, "av23", (0, 2))    # k chunk 3, h-thirds
        pair_sum(2, 0)
        kq1proj(0, 3, "av23", (2, 4))
        scores_exp(6, 0)
        kq1proj(0, 3, "av23", (4, HT))
        scores_exp(7, 0)
        pair_sum(3, 0)
        kq1proj(1, 2, "av01", (0, 2))    # q chunk 2, h-thirds
        scores_exp(8, 0)
        kq1proj(1, 2, "av01", (2, 4))
        scores_exp(9, 0)
        pair_sum(4, 0)
        kq1proj(1, 2, "av01", (4, HT))
        scores_exp(10, 0)
        kq1proj(1, 3, "av23", (0, 2))    # q chunk 3, h-thirds
        scores_exp(11, 0)
        pair_sum(5, 0)
        kq1proj(1, 3, "av23", (2, 4))
        scores_exp(12, 0)
        kq1proj(1, 3, "av23", (4, HT))
        scores_exp(13, 0)
        pair_sum(6, 0)
        v_proj_mm(0, (0, 3))             # v half 0
        scores_exp(14, 0)
        v_proj_mm(0, (3, HT))
        scores_exp(15, 0)

        # ---- phase B: exp stream; av4 trails 2 tiles; PE kept dense in
        # EVERY period (idle periods close the HAM clock gate and halve
        # the whole core's clock); den quads land 2+ periods after their
        # gpsimd pair-sums ----
        scores_exp(0, 1)
        v_transpose(0, 0)
        v_transpose(0, 1)
        scores_exp(1, 1)
        v_transpose(0, 2)
        v_transpose(0, 3)
        v_proj_mm(1, (0, 3))    # v half 1
        pair_sum(0, 1)
        scores_exp(2, 1)
        av4(0)
        v_proj_mm(1, (3, HT))
        scores_exp(3, 1)
        av4(1)
        v_transpose(1, 0)
        v_transpose(1, 1)
        pair_sum(1, 1)
        scores_exp(4, 1)
        av4(2)
        v_transpose(1, 2)
        v_transpose(1, 3)
        scores_exp(5, 1)
        av4(3)
        den4q(0)
        pair_sum(2, 1)
        scores_exp(6, 1)
        av4(4)
        scores_exp(7, 1)
        av4(5)
        den4q(1)
        pair_sum(3, 1)
        scores_exp(8, 1)
        av4(6)
        av_h(14, 0, start=False, stop=False)
        scores_exp(9, 1)
        av4(7)
        den4q(2)
        pair_sum(4, 1)
        scores_exp(10, 1)
        av4(8)
        av_h(15, 0, start=False, stop=False)
        scores_exp(11, 1)
        av4(9)
        den4q(3)
        pair_sum(5, 1)
        scores_exp(12, 1)
        av4(10)
        scores_exp(13, 1)
        av4(11)
        den4q(4)
        pair_sum(6, 1)
        scores_exp(14, 1)
        av4(12)
        scores_exp(15, 1)
        av4(13)  # pav[0] stops here
        den4q(5)
        nc.vector.tensor_copy(out=osb[:, 0:CH], in_=pav[0])
        nc.sync.dma_start(out=o1_d[:, :], in_=osb[:, 0:CH])
        av_h(14, 1, start=False, stop=False)
        den4q(6)                # tiles 12,13
        den4(14)
        av_h(15, 1, start=False, stop=True)
        den4(15, stop=True)
        for i in range(10):
            nc.tensor.matmul(
                warm[:, :P], lhsT=warmT, rhs=warmT,
                start=True, stop=True, skip_group_check=True,
            )

        # ---- epilogue: o2 on DVE and o3 on the now-idle ACT engine run
        # concurrently ----
        nc.vector.tensor_copy(out=osb[:, CH : 2 * CH], in_=pav[1])
        nc.sync.dma_start(out=o2_d[:, :], in_=osb[:, CH : 2 * CH])
        nc.scalar.copy(out=osb[:, 2 * CH : 3 * CH], in_=pden[0])
        nc.sync.dma_start(out=o3_d[:, :], in_=osb[:, 2 * CH : 3 * CH])

    return nc


_NC = None


def kernel(query, key, value, mask, Wq, bq, Wk, bk, Wv, bv):
    global _NC, LAST_RESULT
    bf16 = ml_dtypes.bfloat16
    B = query.shape[0]
    assert B == 8

    if _NC is None:
        _NC = _build()
        _NC.finalize()

    def prepack(w):  # [768, 128] -> [p, t, n] layout [128, 768]
        return np.ascontiguousarray(
            w.reshape(HT, P, P).transpose(1, 0, 2).reshape(P, HT * P).astype(bf16)
        )

    wqk = prepack(np.concatenate([np.asarray(Wq), np.asarray(Wk)], axis=1))
    ict = np.eye(P, dtype=bf16)
    wv = np.ascontiguousarray(
        np.asarray(Wv).reshape(HT, P, D).transpose(1, 0, 2)
        .reshape(P, HT * D).astype(bf16)
    )
    bqk = np.concatenate([np.asarray(bq), np.asarray(bk)]).astype(np.float32)
    bvv = np.concatenate([np.asarray(bv), np.asarray(bv)]).astype(np.float32)
    bkk = np.concatenate([np.asarray(bk), np.asarray(bk)]).astype(np.float32)
    bqq = np.concatenate([np.asarray(bq), np.asarray(bq)]).astype(np.float32)

    def pack_kq(x):  # [2048, 768] -> [128, 4*6*512] SBUF-native
        return np.ascontiguousarray(
            np.asarray(x).reshape(NCH, CH, HT, P).transpose(3, 0, 2, 1)
            .reshape(P, NCH * HT * CH).astype(bf16)
        )

    def pack_v(x):  # wv | [2048, 768] -> [128, 384 + 2*6*1024]
        vp = (np.asarray(x).reshape(2, 2 * CH, HT, P).transpose(3, 0, 2, 1)
              .reshape(P, 2 * HT * 2 * CH).astype(bf16))
        return np.ascontiguousarray(np.concatenate([wv, vp], axis=1))

    in_maps = []
    for b in range(B):
        mb = ((np.asarray(mask[b], np.float32) - 1.0) * 1e9).reshape(NT, P).T
        cst = np.ascontiguousarray(
            np.concatenate(
                [bqk[:, None], bvv[:, None], bkk[:, None], bqq[:, None], mb],
                axis=1,
            )
        ).astype(np.float32)
        in_maps.append(
            {
                "qpk": pack_kq(query[b]),
                "kpk": pack_kq(key[b]),
                "vpk": pack_v(value[b]),
                "wqk": wqk,
                "ict": ict,
                "cst": cst,
            }
        )

    res = run_bass_kernel_spmd(
        _NC,
        in_maps,
        core_ids=list(range(8)),
        trace=bool(os.environ.get("KERNEL_TRACE")),
    )
    LAST_RESULT = res
    out = np.empty((B, S, D), dtype=np.float32)
    for b in range(B):
        o1 = np.asarray(res.results[b]["o1"]).astype(np.float32)  # chunks 0,1
        o2 = np.asarray(res.results[b]["o2"]).astype(np.float32)  # chunks 2,3
        o3 = np.asarray(res.results[b]["o3"]).astype(np.float32)  # denominators
        for ci in range(NCH):
            oh = o1 if ci < 2 else o2
            blk = oh[(ci % 2) * D : (ci % 2) * D + D, :]  # O^T chunk ci
            den = o3[32 * ci, :]
            out[b, ci * CH : (ci + 1) * CH, :] = (blk / den[None, :]).T
    return out


# revision 69
# speedup vs baseline: 1.1860x; 1.1860x over previous
"""Single-head attention (B=8, S=2048, H=768, D=64) on 8 TRN2 NeuronCores.

Data-parallel over batch: core b computes batch element b end to end; no
collectives. Host prepacks Q/K/V into SBUF-native [128, ...] bf16 layouts
(linear DMA: 128 big descriptors per transfer instead of 768 x 2KB lines)
and does the final softmax divide + transpose; the device ships
unnormalized O^T (bf16) plus denominators.

Design rules (from perfetto/NTFF iteration; see git history of the
optimization session for the evidence):
  - The ACT exp stream (32 x [128,1024] @ ~1.11us each, ~35.6us busy) is
    the pacing resource. Makespan ~= first-exp time + ACT busy + tail.
  - ACT is saturated after the first exp, so any PE insertion larger
    than the ~600ns per-period slack delays every later exp permanently:
    all projections are emitted as 2-3 h-granule slices, one per period.
  - The HAM clock gate closes after ~1us of PE idleness and then runs
    the WHOLE core (ACT included) ~20% slower for tens of us: 64 warmup
    matmuls bridge the DMA lead-in, and every phase keeps the PE dense.
    Never leave a >1us PE hole mid-kernel.
  - Input DMA rides the sync queue in exact consumption order
    k0 q0ab q1ab k1 k2 k3 q2 wv q3 v0 v1 (q chunks h-split so their
    projections start early); weights (wqk) ride the scalar
    queue first. DMA completion sems cost ~0.9us (SEM_PROP) on top of
    arrival, and per-transfer issue costs ~0.7us on the issuing engine.
  - Tiles 0,1 run half-width chunk-0 scores+exp so the exp stream starts
    before q chunk 1 is even projected.
  - Softmax denominators: GPSIMD (idle otherwise) sums pth tile pairs
    (~2.1us each, SBUF-only), PE runs 4x-col-packed quads on the pair
    sums two+ periods later; tiles 14,15 use direct quads in the tail.
    DVE must NOT touch the pth pool while ACT writes it (SBUF port
    contention adds ~220ns to every exp) - hence gpsimd.
  - O^T accumulates as av4: both halves of a tile back to back sharing
    one stationary load of the [sk,d] V tile; av half-0 of tiles 14,15
    is injected early so pav[0] stops at av4(13) and o1 ships before the
    last exp. PE-transposes (not DMA_TRANSPOSE: 1.2us issue each!) build
    the [sk,d] V tiles.
"""

import os
from contextlib import ExitStack

import numpy as np
import ml_dtypes

import concourse.bass as bass
import concourse.mybir as mybir
import concourse.tile as tile
from concourse import bacc
from concourse.bass_utils import run_bass_kernel_spmd

S, H, D = 2048, 768, 64
P = 128
NT = S // P      # 16 sk tiles
HT = H // P      # 6 h tiles
CH = 512         # sq chunk = matmul free dim = PSUM bank
NCH = S // CH    # 4
BF = mybir.dt.bfloat16
F32 = mybir.dt.float32
AF = mybir.ActivationFunctionType

LAST_RESULT = None  # BassKernelResults of the most recent run (for test.py)


def _build(debug=False):
    nc = bacc.Bacc()
    qpk_d = nc.declare_dram_parameter("qpk", [P, NCH * HT * CH], BF, isOutput=False)
    kpk_d = nc.declare_dram_parameter("kpk", [P, NCH * HT * CH], BF, isOutput=False)
    # v prepack: {wv | v half0 | v half1}
    VW = HT * 2 * CH  # 6144 elements per v half
    vpk_d = nc.declare_dram_parameter("vpk", [P, HT * D + 2 * VW], BF, isOutput=False)
    wqk_d = nc.declare_dram_parameter("wqk", [P, HT * P], BF, isOutput=False)
    ict_d = nc.declare_dram_parameter("ict", [P, P], BF, isOutput=False)
    cst_d = nc.declare_dram_parameter("cst", [P, 4 + NT], F32, isOutput=False)
    o1_d = nc.declare_dram_parameter("o1", [P, CH], BF, isOutput=True)
    o2_d = nc.declare_dram_parameter("o2", [P, CH], BF, isOutput=True)
    o3_d = nc.declare_dram_parameter("o3", [P, CH], BF, isOutput=True)

    with ExitStack() as ctx:
        tc = ctx.enter_context(tile.TileContext(nc))
        consts = ctx.enter_context(tc.tile_pool(name="consts", bufs=1))
        stages = ctx.enter_context(tc.tile_pool(name="stages", bufs=1))
        persist = ctx.enter_context(tc.tile_pool(name="persist", bufs=1))
        ppool = ctx.enter_context(tc.tile_pool(name="ppool", bufs=2 * NT))
        p2pool = ctx.enter_context(tc.tile_pool(name="p2pool", bufs=14))
        psc = ctx.enter_context(tc.tile_pool(name="psc", bufs=1, space="PSUM"))
        psw = ctx.enter_context(tc.tile_pool(name="psw", bufs=1, space="PSUM"))
        psav = ctx.enter_context(tc.tile_pool(name="psav", bufs=1, space="PSUM"))

        # ---- constants on the scalar queue: wqk first (critical path) ----
        wqk_sb = consts.tile([P, HT * P], BF, tag="wqk")
        nc.scalar.dma_start(out=wqk_sb, in_=wqk_d[:, :])
        cst_sb = consts.tile([P, 4 + NT], F32, tag="cst")
        nc.scalar.dma_start(out=cst_sb, in_=cst_d[:, :])
        ict_sb = consts.tile([P, P], BF, tag="ict")
        nc.scalar.dma_start(out=ict_sb, in_=ict_d[:, :])
        ident_bf = ict_sb
        bvv_sb = cst_sb[:, 1:2]
        bkk_sb = cst_sb[:, 2:3]
        bqq_sb = cst_sb[:, 3:4]
        mb_sb = cst_sb[:, 4 : 4 + NT]
        warmT = consts.tile([P, P], BF, tag="warmT")
        nc.vector.memset(warmT, 0.0)
        ones32 = consts.tile([P, 32], BF, tag="ones32")
        nc.vector.memset(ones32, 1.0)

        def w_qk(h, lo, hi):  # packed q|k weight slice [128, hi-lo]
            return wqk_sb[:, h * P + lo : h * P + hi]

        # ---- input staging: linear transfers on the sync queue in exact
        # consumption order k0,q0,q1,k1,v0(+wvv),k2,k3,v1,q2,q3 ----
        stq = stages.tile([P, NCH, HT * CH], BF, tag="stq")
        stk = stages.tile([P, NCH, HT * CH], BF, tag="stk")
        stv = stages.tile([P, 2, VW], BF, tag="stv")
        wvv_sb = stages.tile([P, HT * D], BF, tag="wvv")
        QW = HT * CH  # 3072 elements per k/q quarter

        def dma_kq(dst, src, c):
            nc.sync.dma_start(out=dst[:, c, :], in_=src[:, c * QW : (c + 1) * QW])

        dma_kq(stk, kpk_d, 0)
        # q chunks 0,1 split in h-halves so projections start ~1us earlier
        HQW = QW // 2
        nc.sync.dma_start(out=stq[:, 0, 0:HQW], in_=qpk_d[:, 0:HQW])
        nc.sync.dma_start(out=stq[:, 0, HQW:QW], in_=qpk_d[:, HQW:QW])
        nc.sync.dma_start(
            out=stq[:, 1, 0:HQW], in_=qpk_d[:, QW : QW + HQW]
        )
        nc.sync.dma_start(
            out=stq[:, 1, HQW:QW], in_=qpk_d[:, QW + HQW : 2 * QW]
        )
        dma_kq(stk, kpk_d, 1)
        dma_kq(stk, kpk_d, 2)
        dma_kq(stk, kpk_d, 3)
        dma_kq(stq, qpk_d, 2)
        WV0 = HT * D
        nc.sync.dma_start(out=wvv_sb, in_=vpk_d[:, 0:WV0])
        dma_kq(stq, qpk_d, 3)
        nc.sync.dma_start(out=stv[:, 0, :], in_=vpk_d[:, WV0 : WV0 + VW])
        nc.sync.dma_start(out=stv[:, 1, :], in_=vpk_d[:, WV0 + VW : WV0 + 2 * VW])

        def w_vv(h):
            return wvv_sb[:, h * D : (h + 1) * D]

        def st_kq(st, h, c):  # [128, 512] h-tile slice of chunk c
            return st[:, c, h * CH : (h + 1) * CH]

        def st_v(h, half, i):  # [128, 512] h-tile slice of chunk 2*half+i
            return stv[:, half, h * 2 * CH + i * CH : h * 2 * CH + (i + 1) * CH]

        # ---- persistent SBUF tensors ----
        qqT_sb = persist.tile([P, S], BF, tag="qqT")
        kkT_sb = persist.tile([P, S], BF, tag="kkT")
        vT2_sb = persist.tile([P, S // 2], BF, tag="vT2")
        vE_sb = persist.tile([P, NT * D], BF, tag="vE")
        osb = persist.tile([P, 3 * CH], BF, tag="osb")  # output staging

        # ---- PE warmup: bridge the whole DMA lead-in ----
        warm = psw.tile([P, CH], F32, tag="aux", name="warm")
        for i in range(64):
            nc.tensor.matmul(
                warm[:, :P],
                lhsT=warmT,
                rhs=warmT,
                start=True,
                stop=True,
                skip_group_check=True,
            )

        # ---- helper blocks ----
        pp_t = {}

        def kq1proj(kind, c, ptag, hs=(0, HT), pool=None):
            """single-chunk projection (kind 0 = k -> kkT, 1 = q -> qqT):
            col-packed concurrent pairs produce native+duplicated partition
            halves of chunk c (h range hs), then one [128,512] bias drain.
            Split across two calls to spread PE load across exp periods."""
            dst = kkT_sb if kind == 0 else qqT_sb
            bias = bkk_sb if kind == 0 else bqq_sb
            wlo = D if kind == 0 else 0
            stx = stk if kind == 0 else stq
            if (kind, c) not in pp_t:
                pp_t[kind, c] = (pool or psav).tile([P, CH], F32, tag=ptag,
                                                    name=f"pp{kind}_{c}")
            pp = pp_t[kind, c]
            for h in range(*hs):
                nc.tensor.matmul(
                    pp[:D, :],
                    lhsT=w_qk(h, wlo, wlo + D),
                    rhs=st_kq(stx, h, c),
                    start=(h == 0),
                    stop=(h == HT - 1),
                    tile_position=(0, 0),
                    skip_group_check=True,
                )
                nc.tensor.matmul(
                    pp[D:, :],
                    lhsT=w_qk(h, wlo, wlo + D),
                    rhs=st_kq(stx, h, c),
                    start=(h == 0),
                    stop=(h == HT - 1),
                    tile_position=(0, D),
                    skip_group_check=True,
                )
            if hs[1] == HT:
                nc.vector.tensor_scalar_add(
                    out=dst[:, c * CH : (c + 1) * CH], in0=pp, scalar1=bias
                )

        pth = {}

        def scores_exp(t, half):
            """scores for sk-tile t over sq half (row group alternates with
            t to hide LDWEIGHTS), exp straight into a bf16 tile."""
            lo, hi = (0, D) if t % 2 == 0 else (D, P)
            ps = psc.tile([P, 2 * CH], F32, tag=f"sc{t % 2}",
                          name=f"ps{t}_{half}")
            for sub in range(2):
                c = 2 * half + sub
                nc.tensor.matmul(
                    ps[:, sub * CH : (sub + 1) * CH],
                    lhsT=kkT_sb[lo:hi, t * P : (t + 1) * P],
                    rhs=qqT_sb[lo:hi, c * CH : (c + 1) * CH],
                    start=True,
                    stop=True,
                    tile_position=(lo, 0),
                    skip_group_check=True,
                )
            pt = ppool.tile([P, 2 * CH], BF, tag="pT", name=f"pt{t}_{half}")
            nc.scalar.activation(
                out=pt,
                in_=ps,
                func=AF.Exp,
                bias=mb_sb[:, t : t + 1],
                scale=0.125,
            )
            pth[t, half] = pt

        pav = {}

        def vE_slice(t):
            b = (t // 8) * 4 + (t % 4)
            half = (t % 8) // 4
            return vE_sb[:, b * P + half * D : b * P + half * D + D]

        def av_h(t, half, start, stop):
            """col-packed concurrent O^T accumulation pair for sk-tile t,
            one half: chunk 2h -> partitions 0:64, chunk 2h+1 -> 64:128."""
            key = "av23" if half == 0 else "av01"
            if half not in pav:
                pav[half] = psav.tile([P, CH], F32, tag=key, name=f"pav{half}")
            vt = vE_slice(t)
            nc.tensor.matmul(
                pav[half][:D, :],
                lhsT=vt,
                rhs=pth[t, half][:, :CH],
                start=start,
                stop=stop,
                tile_position=(0, 0),
                skip_group_check=True,
            )
            nc.tensor.matmul(
                pav[half][D:, :],
                lhsT=vt,
                rhs=pth[t, half][:, CH:],
                start=start,
                stop=stop,
                tile_position=(0, D),
                skip_group_check=True,
            )

        def av4(t):
            """both halves of tile t back to back (shared stationary vt);
            pav0 accumulation order is 0..13 then 14,15 early-injected, so
            start/stop flags are passed per call site."""
            av_h(t, 0, start=(t == 0), stop=(t == 13))
            av_h(t, 1, start=(t == 0), stop=(t == NT - 1))

        pden = [None]

        def _den_quad(rhs_of_ci, start, stop):
            if pden[0] is None:
                pden[0] = psw.tile([P, CH], F32, tag="den", name="pden")
            for ci in range(NCH):
                nc.tensor.matmul(
                    pden[0][32 * ci : 32 * (ci + 1), :],
                    lhsT=ones32[:, :],
                    rhs=rhs_of_ci(ci),
                    start=start,
                    stop=stop,
                    tile_position=(0, 32 * ci),
                    skip_group_check=True,
                )

        p2 = {}

        def pair_sum(k, half):
            """GPSIMD: p2[k,half] = pth[2k,half] + pth[2k+1,half]
            (SBUF only; keeps the denominator adds off PE and DVE)."""
            t2 = p2pool.tile([P, 2 * CH], BF, tag="p2", name=f"p2_{k}_{half}")
            nc.gpsimd.tensor_add(
                out=t2, in0=pth[2 * k, half], in1=pth[2 * k + 1, half]
            )
            p2[k, half] = t2

        def den4q(k):
            """denominator quad on pair-sum tiles (covers tiles 2k, 2k+1)."""
            _den_quad(
                lambda ci: p2[k, ci // 2][:, (ci % 2) * CH : (ci % 2 + 1) * CH],
                start=(k == 0),
                stop=False,
            )

        def den4(t, stop=False):
            """denominator quad on raw pth tiles (used for tiles 12-15)."""
            _den_quad(
                lambda ci: pth[t, ci // 2][:, (ci % 2) * CH : (ci % 2 + 1) * CH],
                start=False,
                stop=stop,
            )

        pv_t = {}

        def v_proj_mm(u, hs=(0, HT)):
            """v projection matmuls (h range hs) + bias for chunk-pair u:
            chunk 2u -> partitions 0:64, chunk 2u+1 -> 64:128 (stacked)."""
            if u not in pv_t:
                pv_t[u] = psw.tile([P, CH], F32,
                                   tag="den" if u == 0 else "aux",
                                   name=f"pv{u}")
            pv = pv_t[u]
            for h in range(*hs):
                nc.tensor.matmul(
                    pv[:D, :],
                    lhsT=w_vv(h),
                    rhs=st_v(h, u, 0),
                    start=(h == 0),
                    stop=(h == HT - 1),
                    tile_position=(0, 0),
                    skip_group_check=True,
                )
                nc.tensor.matmul(
                    pv[D:, :],
                    lhsT=w_vv(h),
                    rhs=st_v(h, u, 1),
                    start=(h == 0),
                    stop=(h == HT - 1),
                    tile_position=(0, D),
                    skip_group_check=True,
                )
            if hs[1] == HT:
                nc.vector.tensor_scalar_add(
                    out=vT2_sb[:, u * CH : (u + 1) * CH], in0=pv,
                    scalar1=bvv_sb,
                )

        def v_transpose(u, j):
            """one [128,128] PE-transpose block of vT2 into vE
            (block b = 4u+j holds sk tiles 8u+j and 8u+j+4)."""
            pt = psw.tile([P, P], BF, tag="den" if u == 0 else "aux",
                          name=f"ptv{u}_{j}")
            nc.tensor.transpose(
                pt,
                in_=vT2_sb[:, u * CH + j * P : u * CH + (j + 1) * P],
                identity=ident_bf,
            )
            b = 4 * u + j
            nc.vector.tensor_copy(out=vE_sb[:, b * P : (b + 1) * P], in_=pt)

        # ---- phase A schedule: every PE insertion is split to fit
        # per-exp-period slack (the ACT stream is saturated; any larger
        # insertion delays every later exp permanently) ----
        kq1proj(0, 0, "av01")            # k chunk 0 (arrives first)
        kq1proj(1, 0, "av23", (0, 3))    # q chunk 0 (h-split transfer)
        kq1proj(1, 0, "av23", (3, HT))
        # tiles 0,1: half-width scores+exp on chunk 0 only, so the exp
        # stream starts ~2us before q chunk 1 is even projected
        ps01 = {}
        for t in (0, 1):
            lo, hi = (0, D) if t % 2 == 0 else (D, P)
            ps01[t] = psc.tile([P, 2 * CH], F32, tag=f"sc{t % 2}",
                               name=f"ps{t}_0")
            pth[t, 0] = ppool.tile([P, 2 * CH], BF, tag="pT", name=f"pt{t}_0")
            nc.tensor.matmul(
                ps01[t][:, 0:CH],
                lhsT=kkT_sb[lo:hi, t * P : (t + 1) * P],
                rhs=qqT_sb[lo:hi, 0:CH],
                start=True, stop=True,
                tile_position=(lo, 0),
                skip_group_check=True,
            )
        # tile 2 chunk-0 rides the otherwise-free "den" PSUM bank so a
        # third early exp fills part of the q-chunk-1 wait
        ps2a = psw.tile([P, CH], F32, tag="den", name="ps2a")
        pth[2, 0] = ppool.tile([P, 2 * CH], BF, tag="pT", name="pt2_0")
        nc.tensor.matmul(
            ps2a,
            lhsT=kkT_sb[0:D, 2 * P : 3 * P],
            rhs=qqT_sb[0:D, 0:CH],
            start=True, stop=True,
            tile_position=(0, 0),
            skip_group_check=True,
        )
        for t in (0, 1):
            nc.scalar.activation(
                out=pth[t, 0][:, 0:CH], in_=ps01[t][:, 0:CH],
                func=AF.Exp, bias=mb_sb[:, t : t + 1], scale=0.125,
            )
        nc.scalar.activation(
            out=pth[2, 0][:, 0:CH], in_=ps2a,
            func=AF.Exp, bias=mb_sb[:, 2:3], scale=0.125,
        )
        kq1proj(1, 1, "aux", (0, 3), pool=psw)   # q chunk 1 (h-split)
        kq1proj(1, 1, "aux", (3, HT), pool=psw)
        ps2b = psc.tile([P, 2 * CH], F32, tag="sc0", name="ps2_0")
        for t in (0, 1):
            lo, hi = (0, D) if t % 2 == 0 else (D, P)
            nc.tensor.matmul(
                ps01[t][:, CH : 2 * CH],
                lhsT=kkT_sb[lo:hi, t * P : (t + 1) * P],
                rhs=qqT_sb[lo:hi, CH : 2 * CH],
                start=True, stop=True,
                tile_position=(lo, 0),
                skip_group_check=True,
            )
        nc.tensor.matmul(
            ps2b[:, CH : 2 * CH],
            lhsT=kkT_sb[0:D, 2 * P : 3 * P],
            rhs=qqT_sb[0:D, CH : 2 * CH],
            start=True, stop=True,
            tile_position=(0, 0),
            skip_group_check=True,
        )
        for t in (0, 1):
            nc.scalar.activation(
                out=pth[t, 0][:, CH : 2 * CH], in_=ps01[t][:, CH : 2 * CH],
                func=AF.Exp, bias=mb_sb[:, t : t + 1], scale=0.125,
            )
        nc.scalar.activation(
            out=pth[2, 0][:, CH : 2 * CH], in_=ps2b[:, CH : 2 * CH],
            func=AF.Exp, bias=mb_sb[:, 2:3], scale=0.125,
        )
        kq1proj(0, 1, "av23", (0, 3))    # k chunk 1
        pair_sum(0, 0)
        kq1proj(0, 1, "av23", (3, HT))
        scores_exp(3, 0)
        kq1proj(0, 2, "av01", (0, 3))    # k chunk 2
        scores_exp(4, 0)
        pair_sum(1, 0)
        kq1proj(0, 2, "av01", (3, HT))
        scores_exp(5, 0)
        kq1proj(0, 3, "av23", (0, 2))    # k chunk 3, h-thirds
        pair_sum(2, 0)
        kq1proj(0, 3, "av23", (2, 4))
        scores_exp(6, 0)
        kq1proj(0, 3, "av23", (4, HT))
        scores_exp(7, 0)
        pair_sum(3, 0)
        kq1proj(1, 2, "av01", (0, 2))    # q chunk 2, h-thirds
        scores_exp(8, 0)
        kq1proj(1, 2, "av01", (2, 4))
        scores_exp(9, 0)
        pair_sum(4, 0)
        kq1proj(1, 2, "av01", (4, HT))
        scores_exp(10, 0)
        kq1proj(1, 3, "av23", (0, 2))    # q chunk 3, h-thirds
        scores_exp(11, 0)
        pair_sum(5, 0)
        kq1proj(1, 3, "av23", (2, 4))
        scores_exp(12, 0)
        kq1proj(1, 3, "av23", (4, HT))
        scores_exp(13, 0)
        pair_sum(6, 0)
        v_proj_mm(0, (0, 3))             # v half 0
        scores_exp(14, 0)
        v_proj_mm(0, (3, HT))
        scores_exp(15, 0)

        # ---- phase B: exp stream; av4 trails 2 tiles; PE kept dense in
        # EVERY period (idle periods close the HAM clock gate and halve
        # the whole core's clock); den quads land 2+ periods after their
        # gpsimd pair-sums ----
        scores_exp(0, 1)
        v_transpose(0, 0)
        v_transpose(0, 1)
        scores_exp(1, 1)
        v_transpose(0, 2)
        v_transpose(0, 3)
        v_proj_mm(1, (0, 3))    # v half 1
        pair_sum(0, 1)
        scores_exp(2, 1)
        av4(0)
        v_proj_mm(1, (3, HT))
        scores_exp(3, 1)
        av4(1)
        v_transpose(1, 0)
        v_transpose(1, 1)
        pair_sum(1, 1)
        scores_exp(4, 1)
        av4(2)
        v_transpose(1, 2)
        v_transpose(1, 3)
        scores_exp(5, 1)
        av4(3)
        den4q(0)
        pair_sum(2, 1)
        scores_exp(6, 1)
        av4(4)
        scores_exp(7, 1)
        av4(5)
        den4q(1)
        pair_sum(3, 1)
        scores_exp(8, 1)
        av4(6)
        av_h(14, 0, start=False, stop=False)
        scores_exp(9, 1)
        av4(7)
        den4q(2)
        pair_sum(4, 1)
        scores_exp(10, 1)
        av4(8)
        av_h(15, 0, start=False, stop=False)
        scores_exp(11, 1)
        av4(9)
        den4q(3)
        pair_sum(5, 1)
        scores_exp(12, 1)
        av4(10)
        scores_exp(13, 1)
        av4(11)
        den4q(4)
        pair_sum(6, 1)
        scores_exp(14, 1)
        av4(12)
        scores_exp(15, 1)
        av4(13)  # pav[0] stops here
        den4q(5)
        nc.vector.tensor_copy(out=osb[:, 0:CH], in_=pav[0])
        nc.sync.dma_start(out=o1_d[:, :], in_=osb[:, 0:CH])
        av_h(14, 1, start=False, stop=False)
        den4q(6)                # tiles 12,13
        den4(14)
        av_h(15, 1, start=False, stop=True)
        den4(15, stop=True)
        for i in range(10):
            nc.tensor.matmul(
                warm[:, :P], lhsT=warmT, rhs=warmT,
                start=True, stop=True, skip_group_check=True,
            )

        # ---- epilogue: o2 on DVE and o3 on the now-idle ACT engine run
        # concurrently ----
        nc.vector.tensor_copy(out=osb[:, CH : 2 * CH], in_=pav[1])
        nc.sync.dma_start(out=o2_d[:, :], in_=osb[:, CH : 2 * CH])
        nc.scalar.copy(out=osb[:, 2 * CH : 3 * CH], in_=pden[0])
        nc.sync.dma_start(out=o3_d[:, :], in_=osb[:, 2 * CH : 3 * CH])

    return nc


_NC = None


def kernel(query, key, value, mask, Wq, bq, Wk, bk, Wv, bv):
    global _NC, LAST_RESULT
    bf16 = ml_dtypes.bfloat16
    B = query.shape[0]
    assert B == 8

    if _NC is None:
        _NC = _build()
        _NC.finalize()

    def prepack(w):  # [768, 128] -> [p, t, n] layout [128, 768]
        return np.ascontiguousarray(
            w.reshape(HT, P, P).transpose(1, 0, 2).reshape(P, HT * P).astype(bf16)
        )

    wqk = prepack(np.concatenate([np.asarray(Wq), np.asarray(Wk)], axis=1))
    ict = np.eye(P, dtype=bf16)
    wv = np.ascontiguousarray(
        np.asarray(Wv).reshape(HT, P, D).transpose(1, 0, 2)
        .reshape(P, HT * D).astype(bf16)
    )
    bqk = np.concatenate([np.asarray(bq), np.asarray(bk)]).astype(np.float32)
    bvv = np.concatenate([np.asarray(bv), np.asarray(bv)]).astype(np.float32)
    bkk = np.concatenate([np.asarray(bk), np.asarray(bk)]).astype(np.float32)
    bqq = np.concatenate([np.asarray(bq), np.asarray(bq)]).astype(np.float32)

    def pack_kq(x):  # [2048, 768] -> [128, 4*6*512] SBUF-native
        return np.ascontiguousarray(
            np.asarray(x).reshape(NCH, CH, HT, P).transpose(3, 0, 2, 1)
            .reshape(P, NCH * HT * CH).astype(bf16)
        )

    def pack_v(x):  # wv | [2048, 768] -> [128, 384 + 2*6*1024]
        vp = (np.asarray(x).reshape(2, 2 * CH, HT, P).transpose(3, 0, 2, 1)
              .reshape(P, 2 * HT * 2 * CH).astype(bf16))
        return np.ascontiguousarray(np.concatenate([wv, vp], axis=1))

    in_maps = []
    for b in range(B):
        mb = ((np.asarray(mask[b], np.float32) - 1.0) * 1e9).reshape(NT, P).T
        cst = np.ascontiguousarray(
            np.concatenate(
                [bqk[:, None], bvv[:, None], bkk[:, None], bqq[:, None], mb],
                axis=1,
            )
        ).astype(np.float32)
        in_maps.append(
            {
                "qpk": pack_kq(query[b]),
                "kpk": pack_kq(key[b]),
                "vpk": pack_v(value[b]),
                "wqk": wqk,
                "ict": ict,
                "cst": cst,
            }
        )

    res = run_bass_kernel_spmd(
        _NC,
        in_maps,
        core_ids=list(range(8)),
        trace=bool(os.environ.get("KERNEL_TRACE")),
    )
    LAST_RESULT = res
    out = np.empty((B, S, D), dtype=np.float32)
    for b in range(B):
        o1 = np.asarray(res.results[b]["o1"]).astype(np.float32)  # chunks 0,1
        o2 = np.asarray(res.results[b]["o2"]).astype(np.float32)  # chunks 2,3
        o3 = np.asarray(res.results[b]["o3"]).astype(np.float32)  # denominators
        for ci in range(NCH):
            oh = o1 if ci < 2 else o2
            blk = oh[(ci % 2) * D : (ci % 2) * D + D, :]  # O^T chunk ci
            den = o3[32 * ci, :]
            out[b, ci * CH : (ci + 1) * CH, :] = (blk / den[None, :]).T
    return out
